# revision 19
# baseline (speedup 1.0000x reference)
"""PointPillars pseudo-image scatter kernel for 8 Trainium2 NeuronCores.

Problem: voxel_features [192000, 64] f32 + batched_indices [192000, 4] i32
(columns: batch, 0, y, x) -> output (16, 64, 400, 400) f32 where
out[b, :, y, x] = voxel row, everything else 0.

Sharding: data-parallel over batches. Core k owns batches (2k, 2k+1) and the
matching 24000 voxel rows. Each core writes its own (2, 64, 400, 400) slice;
no cross-core communication.

Per-core device pipeline (per batch, batches pipelined so batch 1's scatter
descriptor generation hides under batch 0's DMA traffic):
  A. Load the 12000 index rows, compute flat = y*400 + x on DVE (exact in
     fp32 internal arithmetic), then the canvas row r = pi^-1(flat) for the
     pixel permutation pi(base + 16*p + s) = base + s*128 + p (p < 128,
     s < 16, base = 2048-aligned block). pi makes every later access a
     large contiguous DMA descriptor.
  B. Row-scatter: 96 indirect DMAs per batch, each scattering 125 voxel
     rows (256B contiguous each) into a per-batch canvas[r, :]. This is the
     only HW-supported indirect-DMA shape (offsets [P, 1] + in_ [P, D]:
     one descriptor per partition, dst = offset*D). The canvases are extra
     ExternalOutputs: the runtime delivers ExternalOutput buffers
     pre-zeroed (run_bass_kernel_spmd zero-fills them; the bass2jax/PJRT
     path donates zero buffers), so empty pixels are 0 for free.
  C. For each of 79 units of 2048 canvas rows: one contiguous [128, 1024]
     load (partition p holds rows 16p..16p+15; 4KB descriptors), 16 PE
     transposes of [128 px, 64 ch] sub-tiles into a 4-bank PSUM tile
     [64, 2048] where partition c accumulates channel c's 2048 consecutive
     pixels, one PSUM->SBUF drain (DVE/ACT alternating), and one
     [64, 2048] DMA to the channel planes (64 x 8KB descriptors).
     Unit 78 carries only 256 real pixels (160000 = 78*2048 + 256).

Per-core cost model (TimelineSim, with the indirect DMAs modeled by
equivalent plain SWDGE DMAs): ~596 us, DMA-bandwidth-bound (canvas write
6MB + canvas read 84MB + plane write 82MB + input read 12MB at ~360GB/s).
"""

import numpy as np

import concourse.bacc as bacc
import concourse.mybir as mybir
from concourse.bass import IndirectOffsetOnAxis
from concourse.bass_utils import run_bass_kernel_spmd
from concourse.masks import make_identity
from concourse.tile import TileContext
from concourse.tile_rust import add_dep_helper

B, C, NY, NX = 16, 64, 400, 400
PLANE = NY * NX            # 160000
NV = 12000                 # voxels per batch
NB = 2                     # batches per core
NCORES = 8
PPART = 125                # voxel tile partitions
FCOLS = 96                 # 125 * 96 == 12000, exact
NSUP = 79                  # canvas super-blocks of 2048 rows; 79*2048 = 161792
CROWS = NSUP * 2048        # canvas rows per batch

f32 = mybir.dt.float32
i32 = mybir.dt.int32
Alu = mybir.AluOpType


_NC_CACHE = {}


def build_program(stages: str = "ABCLTRO", timing_twin: bool = False):
    key = (stages, timing_twin)
    if key in _NC_CACHE:
        return _NC_CACHE[key]
    nc = _build_program(stages, timing_twin)
    _NC_CACHE[key] = nc
    return nc


def _build_program(stages: str, timing_twin: bool):
    nc = bacc.Bacc(
        "TRN2",
        target_bir_lowering=False,
        debug=False,
        enable_asserts=False,
        num_devices=NCORES,
    )
    vox = nc.dram_tensor("vox", [NB * NV, C], f32, kind="ExternalInput")
    idx = nc.dram_tensor("idx", [NB * NV, 4], i32, kind="ExternalInput")
    out = nc.dram_tensor("out", [NB * C * PLANE], f32, kind="ExternalOutput")
    # per-batch scratch canvases; ExternalOutput => pre-zeroed by the runtime
    canvases = [
        nc.dram_tensor(f"canvas{b}", [CROWS, C], f32, kind="ExternalOutput")
        for b in range(NB)
    ]
    out_bcq = out.ap().rearrange("(bb c q) -> bb c q", bb=NB, c=C)

    with TileContext(nc) as tc:
        with (
            tc.tile_pool(name="sbuf", bufs=1) as pool,
            tc.tile_pool(name="ld", bufs=4) as ldpool,
            tc.tile_pool(name="dr", bufs=4) as drpool,
            tc.tile_pool(name="ps", bufs=2, space="PSUM") as pspool,
        ):
            ident = pool.tile([128, 128], f32, tag="ident")
            make_identity(nc, ident[:])

            for b in range(NB):
                base = b * NV
                idx_t = pool.tile([PPART, FCOLS, 4], i32, tag=f"idx{b}")
                nc.sync.dma_start(
                    out=idx_t[:],
                    in_=idx[base:base + NV, :].rearrange(
                        "(p f) k -> p f k", p=PPART
                    ),
                )
                vox_t = pool.tile([PPART, FCOLS, C], f32, tag=f"vox{b}")
                nc.sync.dma_start(
                    out=vox_t[:],
                    in_=vox[base:base + NV, :].rearrange(
                        "(p f) c -> p f c", p=PPART
                    ),
                )

                # flat = y*400 + x
                flat_t = pool.tile([PPART, FCOLS], i32, tag=f"flat{b}")
                nc.vector.tensor_scalar(
                    out=flat_t[:], in0=idx_t[:, :, 2],
                    scalar1=NX, scalar2=None, op0=Alu.mult,
                )
                nc.vector.tensor_tensor(
                    out=flat_t[:], in0=flat_t[:], in1=idx_t[:, :, 3], op=Alu.add,
                )
                # canvas row r = (flat - ql) + 16*(ql & 127) + (ql >> 7),
                # ql = flat & 2047
                ql_t = pool.tile([PPART, FCOLS], i32, tag=f"ql{b}")
                r_t = pool.tile([PPART, FCOLS], i32, tag=f"r{b}")
                t2_t = pool.tile([PPART, FCOLS], i32, tag=f"t2{b}")
                nc.vector.tensor_scalar(
                    out=ql_t[:], in0=flat_t[:],
                    scalar1=2047, scalar2=None, op0=Alu.bitwise_and,
                )
                # r := flat - ql   (= block*2048)
                nc.vector.tensor_tensor(
                    out=r_t[:], in0=flat_t[:], in1=ql_t[:], op=Alu.subtract,
                )
                # t2 := (ql & 127) * 16
                nc.vector.tensor_scalar(
                    out=t2_t[:], in0=ql_t[:],
                    scalar1=127, scalar2=None, op0=Alu.bitwise_and,
                )
                nc.vector.tensor_scalar(
                    out=t2_t[:], in0=t2_t[:],
                    scalar1=16, scalar2=None, op0=Alu.mult,
                )
                nc.vector.tensor_tensor(
                    out=r_t[:], in0=r_t[:], in1=t2_t[:], op=Alu.add,
                )
                # t2 := ql >> 7
                nc.vector.tensor_scalar(
                    out=t2_t[:], in0=ql_t[:],
                    scalar1=7, scalar2=None, op0=Alu.logical_shift_right,
                )
                nc.vector.tensor_tensor(
                    out=r_t[:], in0=r_t[:], in1=t2_t[:], op=Alu.add,
                )

                # B: row scatters, 125 rows per call
                scatter_insts = []
                for j in range(FCOLS if "B" in stages else 0):
                    if timing_twin:
                        # timing stand-in with identical descriptor shape +
                        # SWDGE path, but a statically-known destination
                        si = nc.gpsimd.dma_start(
                            out=canvases[b].ap()[j * PPART:(j + 1) * PPART, :],
                            in_=vox_t[:, j, :],
                        )
                    else:
                        si = nc.gpsimd.indirect_dma_start(
                            out=canvases[b].ap(),
                            out_offset=IndirectOffsetOnAxis(
                                ap=r_t[:, j:j + 1], axis=0
                            ),
                            in_=vox_t[:, j, :],
                            in_offset=None,
                        )
                    scatter_insts.append(si)

                # C/D: transpose this batch's canvas into channel planes.
                # Unit = 2048 canvas rows -> [128, 1024] load (partition p
                # holds rows 16p..16p+15), 16 PE transposes, one 4-bank PSUM
                # drain, one plane-write DMA. Unit 78 holds 256 real pixels.
                if "C" not in stages:
                    continue
                for t in range(NSUP):
                    full = t < NSUP - 1
                    ld = ldpool.tile([128, 1024], f32, tag="ld")
                    ldi = nc.scalar.dma_start(
                        out=ld[:],
                        in_=canvases[b].ap()[t * 2048:(t + 1) * 2048, :].rearrange(
                            "(p s) c -> p (s c)", p=128
                        ),
                    )
                    if timing_twin:
                        for si in scatter_insts:
                            add_dep_helper(ldi.ins, si.ins, sync=True,
                                           reason="twin canvas RAW")
                    if "L" in stages and "T" not in stages:
                        continue
                    ns = 16 if full else 2   # row-slots used in this unit
                    ps = pspool.tile([64, 128 * ns], f32, tag="ps")
                    for s in range(ns):
                        nc.tensor.transpose(
                            out=ps[:, 128 * s:128 * (s + 1)],
                            in_=ld[:, 64 * s:64 * (s + 1)],
                            identity=ident[:],
                        )
                    if "T" in stages and "R" not in stages:
                        continue
                    dr = drpool.tile([64, 128 * ns], f32, tag="dr")
                    if t % 2 == 0:
                        nc.vector.tensor_copy(out=dr[:], in_=ps[:])
                    else:
                        nc.scalar.activation(
                            dr[:], ps[:], mybir.ActivationFunctionType.Copy
                        )
                    if "R" in stages and "O" not in stages:
                        continue
                    # write-out: partition ch holds channel ch pixels
                    # 2048*t .. 2048*t + 128*ns, contiguous
                    q0 = 2048 * t
                    nc.sync.dma_start(
                        out=out_bcq[b][:, q0:q0 + 128 * ns],
                        in_=dr[:, :],
                    )
    nc.compile()
    return nc


def shard_inputs(voxel_features: np.ndarray, batched_indices: np.ndarray):
    """Split the full inputs into per-core input maps (batches 2k, 2k+1)."""
    vox = np.ascontiguousarray(np.asarray(voxel_features, dtype=np.float32))
    idx = np.ascontiguousarray(np.asarray(batched_indices, dtype=np.int32))
    bcol = idx[:, 0]
    in_maps = []
    for k in range(NCORES):
        sel = np.flatnonzero(bcol // NB == k)
        assert sel.size == NB * NV, (
            f"core {k}: expected {NB * NV} rows, got {sel.size}"
        )
        order = np.argsort(bcol[sel], kind="stable")
        sel = sel[order]
        in_maps.append({"vox": vox[sel], "idx": idx[sel]})
    return in_maps


def run(voxel_features: np.ndarray, batched_indices: np.ndarray, trace: bool = False):
    nc = build_program()
    in_maps = shard_inputs(voxel_features, batched_indices)
    last_err = None
    for _attempt in range(3):  # rare transient NRT_EXEC_UNIT_UNRECOVERABLE
        try:
            res = run_bass_kernel_spmd(
                nc,
                in_maps,
                core_ids=list(range(NCORES)),
                trace=trace,
            )
            break
        except Exception as e:  # noqa: BLE001
            last_err = e
            if "UNRECOVERABLE" not in str(e) and "UNAVAILABLE" not in str(e):
                raise
    else:
        raise last_err
    parts = [r["out"].reshape(NB, C, NY, NX) for r in res.results]
    full = np.concatenate(parts, axis=0)
    return full, res


def kernel(voxel_features: np.ndarray, batched_indices: np.ndarray) -> np.ndarray:
    full, _ = run(voxel_features, batched_indices, trace=False)
    return full
